# revision 1
# baseline (speedup 1.0000x reference)
"""Trainium2 Bass kernel for the 5x5-neighborhood min-L1 loss (nn_NNLoss).

Computation (faithful to the reference):
    gt_pad = pad(ground_truth, rows by nw//2, cols by nh//2, value=-10000)
    norms[b,h,w,s] = sum_c |gt_pad[b,c,h+di,w+dj] - predicted[b,c,h,w]|
                     for s=(di,dj), di in range(nh), dj in range(nw)
    loss = mean over (b,h,w) of min_s norms

Sharding: pure data parallel over the batch dim: 16 images -> 2 per core
across 8 NeuronCores.  Each core returns per-partition partial sums
[128,1]; the host adds them up and divides (the scalar "all-reduce").

Per-core layout (bf16 compute; the DVE 2x tensor_tensor mode has uops
only for bf16 -- fp16 measured 1x on HW):
  - partition dim = 128 H-rows (2 row-blocks cover H=256)
  - free dim fuses (image, channel, W): chunk q = img*C + ch, which
    makes each staging load a single 3-dim DMA
  - row shifts (di) are materialized as `nh` row-shifted bf16 copies of
    the padded ground truth (cross-partition shifts need DMA: DVE lanes
    are per-partition); all `nw` column shifts of one di are computed by
    ONE wide instruction group using a hand-built overlapping-window AP
    (stride-1 shift axis; odd offsets measured penalty-free) against a
    0-stride broadcast view of predicted.
  - per (block, di): sub (DVE, 2x) -> |.| in place (ACT) -> channel-sum
    (2 DVE adds) -> nw running-min ops (DVE).  Free-dim reduce ->
    [128,1] fp32 partials, summed on host.
"""

import os

# The execution path needs the axon PJRT platform; a harness that pins
# JAX_PLATFORMS=cpu would hide the NeuronCores from jax.
if "axon" not in os.environ.get("JAX_PLATFORMS", "axon"):
    os.environ.pop("JAX_PLATFORMS", None)

import numpy as np

B, C, H, W = 16, 3, 256, 256
N_CORES = 8
IPC = B // N_CORES  # images per core
PAD_VAL = -10000.0

_BUILD_CACHE = {}
LAST_EXEC_NS = [None]  # exec_time_ns of the last traced run (for test.py)


def _build(nh, nw):
    """Trace the Bass/Tile program for one core. Returns the Bass object."""
    from contextlib import ExitStack

    import concourse.bacc as bacc
    import concourse.bass as bass  # noqa: F401
    import concourse.tile as tile
    from concourse import mybir
    from concourse.alu_op_type import AluOpType

    f32 = mybir.dt.float32
    # bf16, not fp16: the DVE's 2x tensor_tensor packing mode only has
    # uops for bf16 (fp16 measured at 1x on HW)
    f16 = mybir.dt.bfloat16
    Abs = mybir.ActivationFunctionType.Abs
    Copy = mybir.ActivationFunctionType.Copy

    W_PAD = nh // 2  # pads the W (column) dim -- faithful swap vs torch
    H_PAD = nw // 2  # pads the H (row) dim
    NDI, NDJ = nh, nw  # row / column shift counts
    WP = W + 2 * W_PAD  # padded row width (260)
    Q = C * IPC  # fused (channel, image) chunks: 6
    FD = Q * W  # 1536
    FDP = Q * WP  # 1560
    SW = IPC * W  # 512: per-channel chunk width in the fused free dim
    assert H % 128 == 0
    NBLK = H // 128

    # Bacc (not raw Bass): its compile() splits multi-wait instructions
    # (TRN2 allows at most one sync wait per instruction) among other
    # required lowerings.
    nc = bacc.Bacc("TRN2", target_bir_lowering=False, debug=False)
    pred_d = nc.dram_tensor("predicted", [IPC, C, H, W], f32, kind="ExternalInput")
    gt_d = nc.dram_tensor("ground_truth", [IPC, C, H, W], f32, kind="ExternalInput")
    out_d = nc.dram_tensor("partials", [128, 1], f32, kind="ExternalOutput")

    import bass_rust as _br

    def strided(ap, levels, extra_offset=0):
        """Hand-built free-dim AP on an existing [128, N] view (keeps the
        partition level and base offset; used for the overlapping
        dj-window axis and the 0-stride pred broadcast)."""
        c = ap.copy()
        c.ap = _br.VecI64Pair([list(ap.ap[0])] + [list(l) for l in levels])
        if extra_offset:
            c.offset = c.offset + extra_offset
        return c

    G = NDJ  # all column shifts merged into one wide instruction group

    with tile.TileContext(nc) as tc, ExitStack() as ctx:
        p_stage = ctx.enter_context(tc.tile_pool(name="p_stage", bufs=2))
        p_pool = ctx.enter_context(tc.tile_pool(name="pred", bufs=1))
        g_stage = ctx.enter_context(tc.tile_pool(name="g_stage", bufs=4))
        g_pool = ctx.enter_context(tc.tile_pool(name="gsel", bufs=1))
        d_pool = ctx.enter_context(tc.tile_pool(name="d", bufs=3))
        s_pool = ctx.enter_context(tc.tile_pool(name="s", bufs=2))
        m_pool = ctx.enter_context(tc.tile_pool(name="m", bufs=1))
        r_pool = ctx.enter_context(tc.tile_pool(name="r", bufs=1))

        r_tiles = []
        for b in range(NBLK):
            h0 = 128 * b

            # ---- predicted: one DMA (img-major chunks merge on the DRAM
            # side), fp32 -> bf16 cast on ACT ----
            ps = p_stage.tile([128, FD], f32, tag="p_stage")
            nc.sync.dma_start(
                ps.rearrange("p (q w) -> p q w", q=Q),
                pred_d.ap().rearrange("i c h w -> h (i c) w")[h0 : h0 + 128],
            )
            pt = p_pool.tile([128, FD], f16, tag=f"pred{b}")
            nc.scalar.activation(pt[:, :], ps[:, :], Copy)
            # broadcast view: [p, G(stride 0), Q, W]
            ptb = strided(pt[:, :], [[0, G], [W, Q], [1, W]])

            m = None
            for di in range(NDI):
                # tile row p holds gt_pad row (h0 + p + di)
                p0 = max(0, H_PAD - h0 - di)
                p1 = min(127, H - 1 + H_PAD - h0 - di)
                r0 = h0 + p0 + di - H_PAD
                cnt = p1 - p0 + 1

                gs = g_stage.tile([128, FDP], f32, tag="g_stage")
                gsv = gs.rearrange("p (q w) -> p q w", q=Q)
                # pad columns / pad rows memset BEFORE the DMA (quadrant-
                # aligned partition strips; DMA overwrites the interior);
                # the cast propagates pads into the bf16 tile.
                nc.gpsimd.memset(gsv[:, :, 0:W_PAD], PAD_VAL)
                nc.gpsimd.memset(gsv[:, :, W_PAD + W : WP], PAD_VAL)
                if p0 > 0:
                    nc.gpsimd.memset(gs[0:32, :], PAD_VAL)
                if p1 < 127:
                    nc.gpsimd.memset(gs[96:128, :], PAD_VAL)
                # alternate DMA issue between the sync (HWDGE) and gpsimd
                # (SWDGE) sequencers: descriptor generation is serial per
                # sequencer (~5.5us per 768-descriptor load) and was half
                # the kernel span when all loads went through sync
                dma_eng = nc.sync if di % 2 == 0 else nc.gpsimd
                dma_eng.dma_start(
                    gsv[p0 : p1 + 1, :, W_PAD : W_PAD + W],
                    gt_d.ap().rearrange("i c h w -> h (i c) w")[r0 : r0 + cnt],
                )
                g0 = g_pool.tile([128, FDP], f16, tag=f"g{b}_{di}")
                nc.scalar.activation(g0[:, :], gs[:, :], Copy)

                # ---- all NDJ column shifts in ONE instruction group ----
                # gt operand: overlapping window axis [1, G] (odd offsets
                # measured penalty-free on HW)
                gt_op = strided(g0[:, :], [[1, G], [WP, Q], [1, W]])
                dG = d_pool.tile([128, G * FD], f16, tag="d")
                d_out = strided(dG[:, :], [[FD, G], [W, Q], [1, W]])
                nc.vector.tensor_sub(d_out, gt_op, ptb)
                # |d| in place on ACT (1x but off the DVE critical path);
                # two halves so downstream adds can start sooner
                half = (G // 2) * FD
                if half:
                    nc.scalar.activation(dG[:, 0:half], dG[:, 0:half], Abs)
                nc.scalar.activation(dG[:, half : G * FD], dG[:, half : G * FD], Abs)
                # channel sum: chunks are img-major (q = i*C + c), so the
                # c-slices are [G, IPC, W] strided views at offset c*W
                CW = C * W
                dc = [
                    strided(dG[:, :], [[FD, G], [CW, IPC], [1, W]], c * W)
                    for c in range(C)
                ]
                s01 = s_pool.tile([128, G * SW], f16, tag="s01")
                s01v = strided(s01[:, :], [[SW, G], [W, IPC], [1, W]])
                nc.vector.tensor_add(s01v, dc[0], dc[1])
                sG = s_pool.tile([128, G * SW], f16, tag="sG")
                sGv = strided(sG[:, :], [[SW, G], [W, IPC], [1, W]])
                nc.vector.tensor_add(sGv, s01v, dc[2])
                # running min, [128, SW] slices (wide MIN measured slow)
                sl = [sG[:, g * SW : (g + 1) * SW] for g in range(G)]
                k = 0
                if m is None:
                    m = m_pool.tile([128, SW], f16, tag=f"m{b}")
                    if G >= 2:
                        nc.vector.tensor_tensor(m, sl[0], sl[1], AluOpType.min)
                        k = 2
                    else:
                        nc.vector.tensor_copy(m, sl[0])
                        k = 1
                for g in range(k, G):
                    nc.vector.tensor_tensor(m, m, sl[g], AluOpType.min)

            r = r_pool.tile([128, 1], f32, tag=f"r{b}")
            nc.vector.tensor_reduce(r, m, mybir.AxisListType.X, AluOpType.add)
            r_tiles.append(r)

        tot = r_tiles[0]
        for b in range(1, NBLK):
            nxt = r_pool.tile([128, 1], f32, tag=f"tot{b}")
            nc.vector.tensor_add(nxt, tot, r_tiles[b])
            tot = nxt
        nc.sync.dma_start(out_d.ap()[:, :], tot)

    nc.compile()
    return nc


def _get_nc(nh, nw):
    key = (nh, nw)
    if key not in _BUILD_CACHE:
        _BUILD_CACHE[key] = _build(nh, nw)
    return _BUILD_CACHE[key]


def _setup_trace():
    """Register the axon NTFF profile hook (the image's antenv lacks
    axon_hooks) and stub the artifact upload so trace=True works."""
    import sys
    import types

    from concourse import bass_utils

    try:
        import antenv.axon_hooks  # noqa: F401
    except ImportError:
        try:
            import trn_agent_boot.trn_boot as tb

            hook = tb._ntff_profile_via_ctypes("/opt/axon/libaxon_pjrt.so")
            mod = types.ModuleType("antenv.axon_hooks")
            mod.get_axon_ntff_profile_hook = lambda: hook
            sys.modules["antenv.axon_hooks"] = mod
        except Exception as e:  # profiling is best-effort
            print(f"ntff hook setup failed: {e}")
            return False
    bass_utils.upload_artifacts = lambda tmpdir: f"local:{tmpdir}"
    return True


def kernel(predicted, ground_truth, nh=5, nw=5):
    from concourse import bass_utils

    nh, nw = int(nh), int(nw)
    pred = np.ascontiguousarray(np.asarray(predicted, dtype=np.float32))
    gt = np.ascontiguousarray(np.asarray(ground_truth, dtype=np.float32))
    assert pred.shape == (B, C, H, W) and gt.shape == (B, C, H, W)

    nc = _get_nc(nh, nw)
    in_maps = [
        {
            "predicted": pred[k * IPC : (k + 1) * IPC],
            "ground_truth": gt[k * IPC : (k + 1) * IPC],
        }
        for k in range(N_CORES)
    ]
    trace = bool(int(os.environ.get("NNLOSS_TRACE", "0")))
    if trace:
        trace = _setup_trace()
    res = bass_utils.run_bass_kernel_spmd(
        nc, in_maps, list(range(N_CORES)), trace=trace
    )
    LAST_EXEC_NS[0] = res.exec_time_ns
    total = 0.0
    for r in res.results:
        total += float(np.asarray(r["partials"], dtype=np.float64).sum())
    return np.float32(total / (B * H * W))



# revision 8
# speedup vs baseline: 1.3472x; 1.3472x over previous
"""Trainium2 Bass kernel for the 5x5-neighborhood min-L1 loss (nn_NNLoss).

Computation (faithful to the reference):
    gt_pad = pad(ground_truth, rows by nw//2, cols by nh//2, value=-10000)
    norms[b,h,w,s] = sum_c |gt_pad[b,c,h+di,w+dj] - pred[b,c,h,w]|
                     for s=(di,dj), di in range(nh), dj in range(nw)
    loss = mean over (b,h,w) of min_s norms

Sharding: pure data parallel over the batch dim: 16 images -> 2 per core
across 8 NeuronCores.  Each core returns per-partition partial sums
[128,1]; the host adds them up and divides (the scalar "all-reduce").

Per-core layout (v2 -- single row-block, 2 rows per partition):
  - partition p holds image rows {2p, 2p+1} (sub-row s in {0,1}); free dim
    is [q=(img,chan), s, w].  This makes every HBM load ONE dma with 2KB
    contiguous descriptors (vs 1KB row-per-partition), and the whole H=256
    fits a single 128-partition block.
  - ground_truth is loaded ONCE (SWDGE dma casts f32->bf16 in flight);
    the nh row shifts decompose into partition shifts k in {-1,0,+1}
    (built from the base tile by two SBUF->SBUF DMAs, 6KB descriptors)
    plus a sub-row select s'.
  - NO pad values are materialized: out-of-range column shifts are
    excluded from the running min by restricting the min-update APs to
    the valid w range, and out-of-range rows by memsetting the boundary
    partition of the shifted tiles to +10000 (|10000 - pred| can never
    win the min: real sums are < ~30).
  - per (di, s) unit: one wide sub (DVE, all nw column shifts via an
    overlapping-window AP at 2x bf16) -> |.| in place (ACT) -> channel
    sum (2 DVE adds) -> running-min updates (DVE, w-restricted APs).
"""

import os

# The execution path needs the axon PJRT platform; a harness that pins
# JAX_PLATFORMS=cpu would hide the NeuronCores from jax.
if "axon" not in os.environ.get("JAX_PLATFORMS", "axon"):
    os.environ.pop("JAX_PLATFORMS", None)

import numpy as np

B, C, H, W = 16, 3, 256, 256
N_CORES = 8
IPC = B // N_CORES  # images per core
PAD_BIG = 10000.0  # stand-in for the reference's pad: never wins the min

_BUILD_CACHE = {}
LAST_EXEC_NS = [None]  # exec_time_ns of the last traced run (for test.py)
LAST_RES = [None]  # full BassKernelResults of the last run (for analysis)


def _build(nh, nw):
    """Trace the Bass/Tile program for one core. Returns the Bass object."""
    from contextlib import ExitStack

    import concourse.bacc as bacc
    import concourse.bass as bass  # noqa: F401
    import concourse.tile as tile
    from concourse import mybir
    from concourse.alu_op_type import AluOpType

    f32 = mybir.dt.float32
    # bf16, not fp16: the DVE's 2x tensor_tensor packing mode only has
    # uops for bf16 (fp16 measured at 1x on HW)
    f16 = mybir.dt.bfloat16
    Abs = mybir.ActivationFunctionType.Abs
    Copy = mybir.ActivationFunctionType.Copy

    # Faithful to the reference's crossed pad/shift pairing:
    #   row shifts   di in range(nh), offset d  = di - nw//2
    #   col shifts   g  in range(nw), offset    = g  - nh//2
    H_PAD = nw // 2
    W_PAD = nh // 2
    NDI, G = nh, nw
    S = 2  # rows packed per partition
    assert H == 128 * S
    Q = C * IPC  # fused (img, chan) chunks: 6
    SW = S * W  # 512
    FDW = Q * SW  # 3072: data columns of the packed tiles
    MARG = W_PAD  # margin columns so the window AP stays in-bounds
    GQW = G * Q * W  # 7680: one (di, s) diff tensor
    GIW = G * IPC * W  # 2560: one (di, s) channel-summed tensor
    IW = IPC * W  # 512: running-min tensor

    # (di, s) -> (partition shift k, source sub-row s'): the target row
    # 2p + s + (di - H_PAD) lives at partition p + k, sub-row s'
    def shift_of(di, s):
        idx = s + di - H_PAD
        return idx // S, idx % S

    units = [(di, s) for di in range(NDI) for s in range(S)]
    ks_needed = sorted({shift_of(di, s)[0] for di, s in units})

    # valid output-w range for column shift g (shifts reading outside the
    # row are excluded from the min -- the reference's pad value loses
    # every min it enters, so exclusion is equivalent)
    def wrange(g):
        lo = max(0, W_PAD - g)
        hi = W + min(0, W_PAD - g)
        return lo, hi

    # Bacc (not raw Bass): its compile() splits multi-wait instructions
    # (TRN2 allows at most one sync wait per instruction) among other
    # required lowerings.
    nc = bacc.Bacc("TRN2", target_bir_lowering=False, debug=False)
    pred_d = nc.dram_tensor("predicted", [IPC, C, H, W], f32, kind="ExternalInput")
    gt_d = nc.dram_tensor("ground_truth", [IPC, C, H, W], f32, kind="ExternalInput")
    out_d = nc.dram_tensor("partials", [128, 1], f32, kind="ExternalOutput")

    import bass_rust as _br

    def strided(ap, levels, extra_offset=0):
        """Hand-built free-dim AP on an existing [128, N] view (keeps the
        partition level and base offset)."""
        c = ap.copy()
        c.ap = _br.VecI64Pair([list(ap.ap[0])] + [list(l) for l in levels])
        if extra_offset:
            c.offset = c.offset + extra_offset
        return c

    with tile.TileContext(nc) as tc, ExitStack() as ctx:
        g_pool = ctx.enter_context(tc.tile_pool(name="gt", bufs=1))
        p_pool = ctx.enter_context(tc.tile_pool(name="pred", bufs=1))
        d_pool = ctx.enter_context(tc.tile_pool(name="d", bufs=3))
        s_pool = ctx.enter_context(tc.tile_pool(name="s", bufs=2))
        m_pool = ctx.enter_context(tc.tile_pool(name="m", bufs=1))
        r_pool = ctx.enter_context(tc.tile_pool(name="r", bufs=1))

        # ---- ground truth: one SWDGE dma, f32->bf16 cast in flight,
        # 2KB descriptors (2 contiguous rows per partition) ----
        gt_t = {}
        gt_t[0] = g_pool.tile(
            [128, MARG + FDW + MARG], f16, tag="gt0", name="gt0"
        )
        gt_src = gt_d.ap().rearrange("i c (p s) w -> p (i c) (s w)", s=S)
        nc.gpsimd.dma_start(
            gt_t[0][:, MARG : MARG + FDW].rearrange("p (q x) -> p q x", q=Q),
            gt_src,
        )

        # ---- predicted: HWDGE f32 load + ACT cast ----
        p_stage = p_pool.tile([128, FDW], f32, tag="p_stage")
        nc.scalar.dma_start(
            p_stage.rearrange("p (q x) -> p q x", q=Q),
            pred_d.ap().rearrange("i c (p s) w -> p (i c) (s w)", s=S),
        )
        pred_t = p_pool.tile([128, FDW], f16, tag="pred")
        nc.scalar.activation(pred_t[:, :], p_stage[:, :], Copy)

        # ---- partition-shifted gt copies (SBUF->SBUF, 6KB descriptors);
        # boundary partitions + margins take PAD_BIG so out-of-range rows
        # lose every min ----
        for k in ks_needed:
            if k == 0:
                t = gt_t[0]
                nc.vector.memset(t[:, 0:MARG], PAD_BIG)
                nc.vector.memset(t[:, MARG + FDW :], PAD_BIG)
                continue
            t = g_pool.tile(
                [128, MARG + FDW + MARG], f16, tag=f"gt{k}", name=f"gt{k}"
            )
            # compute-engine partition access must be quadrant-aligned:
            # memset a whole 32-partition strip first, then let the shift
            # DMA overwrite the valid interior
            if k < 0:
                nc.vector.memset(t[0:32, :], PAD_BIG)
                nc.sync.dma_start(
                    t[-k:128, MARG : MARG + FDW],
                    gt_t[0][0 : 128 + k, MARG : MARG + FDW],
                )
            else:
                nc.vector.memset(t[96:128, :], PAD_BIG)
                nc.sync.dma_start(
                    t[0 : 128 - k, MARG : MARG + FDW],
                    gt_t[0][k:128, MARG : MARG + FDW],
                )
            nc.vector.memset(t[:, 0:MARG], PAD_BIG)
            nc.vector.memset(t[:, MARG + FDW :], PAD_BIG)
            gt_t[k] = t

        # process units that only need the unshifted tile first, so the
        # compute pipeline starts as soon as the gt load lands
        units.sort(key=lambda u: abs(shift_of(*u)[0]))

        m_tiles = {}
        for di, s in units:
            k, sp = shift_of(di, s)
            src = gt_t[k]

            # ---- wide sub: all G column shifts in one 2x bf16 DVE op ----
            # gt operand: window axis [1, G] on the packed row sp
            gt_op = strided(
                src[:, :], [[1, G], [SW, Q], [1, W]], MARG + sp * W - W_PAD
            )
            pr_op = strided(pred_t[:, :], [[0, G], [SW, Q], [1, W]], s * W)
            d = d_pool.tile([128, GQW], f16, tag="d")
            d_out = strided(d[:, :], [[Q * W, G], [W, Q], [1, W]])
            nc.vector.tensor_sub(d_out, gt_op, pr_op)

            # ---- |d| in place on ACT ----
            nc.scalar.activation(d[:, :], d[:, :], Abs)

            # ---- channel sum: q = i*C + c, so c-slices are strided views
            CW = C * W
            dc = [
                strided(d[:, :], [[Q * W, G], [CW, IPC], [1, W]], c * W)
                for c in range(C)
            ]
            s01 = s_pool.tile([128, GIW], f16, tag="s01")
            v01 = strided(s01[:, :], [[IW, G], [W, IPC], [1, W]])
            nc.vector.tensor_add(v01, dc[0], dc[1])
            sG = s_pool.tile([128, GIW], f16, tag="sG")
            vG = strided(sG[:, :], [[IW, G], [W, IPC], [1, W]])
            nc.vector.tensor_add(vG, v01, dc[2])

            # ---- running min with w-restricted updates ----
            def mview(t, g, lo, hi):
                return strided(t[:, :], [[W, IPC], [1, hi - lo]], g * IW + lo)

            if s not in m_tiles:
                # first unit per s is di-center (k=0): its g=center slice
                # is fully valid -- init by copy, then restricted updates
                m = m_pool.tile([128, IW], f16, tag=f"m{s}")
                m_tiles[s] = m
                nc.scalar.activation(
                    strided(m[:, :], [[W, IPC], [1, W]]),
                    mview(sG, W_PAD, 0, W),
                    Copy,
                )
                order = [g for g in range(G) if g != W_PAD]
            else:
                m = m_tiles[s]
                order = list(range(G))
            for g in order:
                lo, hi = wrange(g)
                mv = strided(m[:, :], [[W, IPC], [1, hi - lo]], lo)
                nc.vector.tensor_tensor(mv, mv, mview(sG, g, lo, hi), AluOpType.min)

        # ---- free-dim reduce -> [128,1] fp32 partials ----
        r0 = r_pool.tile([128, 1], f32, tag="r0")
        nc.vector.tensor_reduce(r0, m_tiles[0], mybir.AxisListType.X, AluOpType.add)
        r1 = r_pool.tile([128, 1], f32, tag="r1")
        nc.vector.tensor_reduce(r1, m_tiles[1], mybir.AxisListType.X, AluOpType.add)
        tot = r_pool.tile([128, 1], f32, tag="tot")
        nc.vector.tensor_add(tot, r0, r1)
        nc.sync.dma_start(out_d.ap()[:, :], tot)

    nc.compile()
    return nc


def _get_nc(nh, nw):
    key = (nh, nw)
    if key not in _BUILD_CACHE:
        _BUILD_CACHE[key] = _build(nh, nw)
    return _BUILD_CACHE[key]


def _setup_trace():
    """Register the axon NTFF profile hook (the image's antenv lacks
    axon_hooks) and stub the artifact upload so trace=True works."""
    import sys
    import types

    from concourse import bass_utils

    try:
        import antenv.axon_hooks  # noqa: F401
    except ImportError:
        try:
            import trn_agent_boot.trn_boot as tb

            hook = tb._ntff_profile_via_ctypes("/opt/axon/libaxon_pjrt.so")
            mod = types.ModuleType("antenv.axon_hooks")
            mod.get_axon_ntff_profile_hook = lambda: hook
            sys.modules["antenv.axon_hooks"] = mod
        except Exception as e:  # profiling is best-effort
            print(f"ntff hook setup failed: {e}")
            return False
    bass_utils.upload_artifacts = lambda tmpdir: f"local:{tmpdir}"
    return True


def kernel(predicted, ground_truth, nh=5, nw=5):
    from concourse import bass_utils

    nh, nw = int(nh), int(nw)
    pred = np.ascontiguousarray(np.asarray(predicted, dtype=np.float32))
    gt = np.ascontiguousarray(np.asarray(ground_truth, dtype=np.float32))
    assert pred.shape == (B, C, H, W) and gt.shape == (B, C, H, W)

    nc = _get_nc(nh, nw)
    in_maps = [
        {
            "predicted": pred[k * IPC : (k + 1) * IPC],
            "ground_truth": gt[k * IPC : (k + 1) * IPC],
        }
        for k in range(N_CORES)
    ]
    trace = bool(int(os.environ.get("NNLOSS_TRACE", "0")))
    if trace:
        trace = _setup_trace()
    res = bass_utils.run_bass_kernel_spmd(
        nc, in_maps, list(range(N_CORES)), trace=trace
    )
    LAST_EXEC_NS[0] = res.exec_time_ns
    LAST_RES[0] = res
    total = 0.0
    for r in res.results:
        total += float(np.asarray(r["partials"], dtype=np.float64).sum())
    return np.float32(total / (B * H * W))
